# revision 8
# baseline (speedup 1.0000x reference)
"""Multi-head attention V2 kernel for Trainium2 (8 NeuronCores).

Problem shapes (hardcoded): x [4, 2048, 512] f32, Wq [512, 4096], Wv unused,
Wp [4096, 512], bp [512].  Reference math (note: V uses the Q projection):
    q = v = (x @ Wq) -> [B, H, N, D] with H=8, head dim = D = 512
    S = q @ x^T / sqrt(D);  P = softmax(S, -1);  out = (P @ v) @ Wp + bp

Sharding: core = (batch b, head-group hg) with 2 groups of 4 heads.
Each core gets x[b]^T and the Wq columns / Wp rows of its 4 heads, computes
its partial output [N, D]; host sums the two head-group partials per batch
and adds the bias.

Per-core kernel. Matmul inputs fp16 with fp32 PSUM accumulation, except:
the last KS8 feature k-tiles of the scores contraction and the last MT8
token m-tiles of the AV contraction run as fp8-e4m3 DoubleRow pairs (2
k-tiles per PE instruction, 2x MAC rate), trading a controlled amount of
quantization noise (rel err ~1.6e-2 vs the 2e-2 gate) for ~80us of PE time.

  xT [512, 2048] fp16, xT8 (fp8 copy of the KS8 tail k-tiles) and Wq
  resident in SBUF.
  Per head h (stage B emitted one chunk early for pipelining):
    q_h[m, j] = x Wq_h on PE (lhsT=xT tile, rhs=Wq block) -> PSUM -> qn fp16
      (+ fp8 casts qn8 of the MT8 tail token tiles)
    qT via DMA xbar transposes of qn tiles: fp16 k-tiles into qT, the KS8
      tail k-tiles through a small fp16 scratch then DVE-cast to qT8 fp8.
      (head 0 chunk 0 has no latency slack: its qT/qT8 computed on PE.)
    per 512-column chunk c of n:
      S^T[m, n]: per m-tile, KT-KS8 fp16 matmuls + KS8/2 fp8 DoubleRow -> PSUM
      expS = exp(S^T/sqrt(D)) (ScalarE, PSUM -> SBUF fp16; fp8 for MT8 tail)
      den: DVE running-sum of the 16 exp tiles (fp16 partials), then ONE
        ones[128,128] fp16 matmul sums partitions (f32 PSUM) + broadcasts
      rcpB = 1/den (DVE reciprocal_approx_fast, fp16)
      U^T[d, n]: lhsT=qn tiles fp16 + fp8 DoubleRow pairs for the MT8 tail
      outT = U^T * rcpB (DVE) -> SBUF fp16
  y[n, e] = sum_h outT_h^T @ Wp_h (interleaved into head 3's chunks),
  emitted fp16; host upcasts, sums the 2 head-group partials, adds bias.
Softmax skips the max-subtraction: scores are q.x/sqrt(512) with |s| < ~6,
so exp is safely in range and the result is mathematically identical.
"""

import sys

sys.path.insert(0, "/opt/trn_rl_repo")

import numpy as np
import ml_dtypes

B, N, D, H = 4, 2048, 512, 8
NCORES = 8
HG = 2            # head groups (cores per batch)
HPG = H // HG     # heads per core
JW = HPG * D      # per-core Wq column count / Wp row count (2048)
KT = D // 128     # k-tiles over feature dim (4)
NT = N // 128     # partition tiles over tokens (16)
NCHUNK = 4        # n split into 4 chunks of 512
CW = N // NCHUNK  # chunk width (512)
INV_SQRT_D = 1.0 / float(np.sqrt(D))

# fp8 DoubleRow fractions (even): KS8 of the KT feature k-tiles in the
# scores contraction, MT8 of the NT token m-tiles in the AV contraction.
KS8 = 0
MT8 = 12
KCUT = KT - KS8   # first fp8 feature k-tile
MCUT = NT - MT8   # first fp8 token m-tile

_state = {}


def _build():
    import concourse.bass as bass
    import concourse.mybir as mybir
    import concourse.tile as tile
    from concourse import bacc

    f32 = mybir.dt.float32
    f16 = mybir.dt.float16
    f8 = mybir.dt.float8e4
    DR = mybir.MatmulPerfMode.DoubleRow

    nc = bacc.Bacc("TRN2", target_bir_lowering=False)

    xT_d = nc.dram_tensor("xt", [D, N], f16, kind="ExternalInput")
    wq_d = nc.dram_tensor("wq", [D, JW], f16, kind="ExternalInput")
    wp_d = nc.dram_tensor("wp", [JW, D], f16, kind="ExternalInput")
    y_d = nc.dram_tensor("y", [N, D], f16, kind="ExternalOutput")

    with tile.TileContext(nc) as tc:
        with (
            tc.tile_pool(name="const", bufs=1) as cpool,
            tc.tile_pool(name="qt", bufs=1) as qt_pool,
            tc.tile_pool(name="qn", bufs=2) as qn_pool,
            tc.tile_pool(name="exps", bufs=2) as exps_pool,
            tc.tile_pool(name="outt", bufs=1) as outt_pool,
            tc.tile_pool(name="dacc", bufs=2) as dacc_pool,
            tc.tile_pool(name="rcp", bufs=2) as rcp_pool,
            tc.tile_pool(name="ysb", bufs=2) as y_pool,
            tc.tile_pool(name="ps_stage", bufs=2, space="PSUM") as ps_stage,
            tc.tile_pool(name="ps_scores", bufs=3, space="PSUM") as ps_scores,
            tc.tile_pool(name="ps_av", bufs=2, space="PSUM") as ps_av,
            tc.tile_pool(name="ps_bcast", bufs=1, space="PSUM") as ps_bcast,
        ):
            # ---- resident inputs ----
            xT = cpool.tile([128, KT, N], f16, name="xT")
            wq = cpool.tile([128, KT, JW], f16, name="wq")
            wp = cpool.tile([128, JW // 128, D], f16, name="wp")
            # critical first wave: head-0 b_tiles need xT chunk cols + full
            # head-0 wq block; split in halves so 16 DMA engines share it
            for k in range(KT):
                for u in range(2):
                    nc.sync.dma_start(
                        xT[:, k, u * 256 : (u + 1) * 256],
                        xT_d[k * 128 : (k + 1) * 128, u * 256 : (u + 1) * 256],
                    )
            for k in range(KT):
                for u in range(2):
                    nc.sync.dma_start(
                        wq[:, k, u * 256 : (u + 1) * 256],
                        wq_d[k * 128 : (k + 1) * 128, u * 256 : (u + 1) * 256],
                    )
            for c in range(1, NCHUNK):
                for k in range(KT):
                    nc.sync.dma_start(
                        xT[:, k, c * CW : (c + 1) * CW],
                        xT_d[k * 128 : (k + 1) * 128, c * CW : (c + 1) * CW],
                    )

            def load_noncritical():
                for h in range(1, HPG):
                    for k in range(KT):
                        nc.sync.dma_start(
                            wq[:, k, h * D : (h + 1) * D],
                            wq_d[k * 128 : (k + 1) * 128, h * D : (h + 1) * D],
                        )
                for j in range(JW // 128):
                    nc.sync.dma_start(wp[:, j, :], wp_d[j * 128 : (j + 1) * 128, :])

            load_noncritical()

            ones_col = cpool.tile([128, 1], f16, name="ones_col")
            nc.vector.memset(ones_col[:, :], 1.0)
            # touch Exp once during the input-DMA wait so the ACT table-set
            # load is off the first chunk's critical path
            nc.scalar.activation(
                ones_col[:, :], ones_col[:, :],
                mybir.ActivationFunctionType.Exp, scale=0.0,
            )
            nc.vector.memset(ones_col[:, :], 1.0)
            ones128 = cpool.tile([128, 128], f16, name="ones128")
            nc.vector.memset(ones128[:, :], 1.0)

            def emit_stage_b(h):
                # stage B: q_h [m, j] token-major on PE; qT/qT8 via DMA xbar
                j0 = h * D
                qTs = [
                    qt_pool.tile([128, KT, CW], f16, name=f"qT{c}", tag=f"qT{c}")
                    for c in range(NCHUNK)
                ]
                qn = qn_pool.tile([128, NT, D], f16, name="qn", tag="qn")
                qn8 = None
                if MT8:
                    qn8 = qn_pool.tile([128, MT8, D], f8, name="qn8", tag="qn8")

                def b_tile(mt):
                    ps = ps_stage.tile([128, D], f32, name="ps_b", tag="stage")
                    for k in range(KT):
                        nc.tensor.matmul(
                            ps[:, :],
                            lhsT=xT[:, k, mt * 128 : (mt + 1) * 128],
                            rhs=wq[:, k, j0 : j0 + D],
                            start=(k == 0),
                            stop=(k == KT - 1),
                        )
                    if mt % 2 == 0:
                        nc.scalar.copy(qn[:, mt, :], ps[:, :])
                    else:
                        nc.vector.tensor_copy(qn[:, mt, :], ps[:, :])
                    if MT8 and mt >= MCUT:
                        nc.vector.tensor_copy(qn8[:, mt - MCUT, :], qn[:, mt, :])
                    # one xbar transpose per mt: [128, 512] -> 3D block of qT
                    if h != 0 or mt >= CW // 128:
                        nc.sync.dma_start_transpose(
                            qTs[mt // 4][:, :, (mt % 4) * 128 : (mt % 4 + 1) * 128],
                            qn[:, mt, :],
                        )

                if h == 0:
                    # head 0 has no prior work to hide the transpose latency
                    # behind: compute its first qT chunk directly on the PE.
                    for mt in range(4):
                        b_tile(mt)
                    for jt in range(KT):
                        ps = ps_stage.tile([128, CW], f32, name="ps_a", tag="stage")
                        for k in range(KT):
                            nc.tensor.matmul(
                                ps[:, :],
                                lhsT=wq[:, k, jt * 128 : (jt + 1) * 128],
                                rhs=xT[:, k, 0:CW],
                                start=(k == 0),
                                stop=(k == KT - 1),
                            )
                        nc.scalar.copy(qTs[0][:, jt, :], ps[:, :])
                    for mt in range(4, NT):
                        b_tile(mt)
                else:
                    for mt in range(NT):
                        b_tile(mt)
                return qTs, qn, qn8

            outTs = []
            pending = emit_stage_b(0)
            for h in range(HPG):
                qTs, qn, qn8 = pending
                outT = outt_pool.tile(
                    [128, KT, N], f16, name=f"outT{h}", tag=f"outT{h}"
                )
                outTs.append(outT)

                for c in range(NCHUNK):
                    # emit the next head's stage B ahead of this head's last
                    # chunk so its transposes finish before the head boundary
                    if c == NCHUNK - 1 and h + 1 < HPG:
                        pending = emit_stage_b(h + 1)
                    n0 = c * CW
                    # ---- scores S^T[m, n-chunk] + exp + den partials ----
                    expS = exps_pool.tile(
                        [128, MCUT, CW], f16, name="expS", tag="expS"
                    )
                    expS8 = None
                    if MT8:
                        expS8 = exps_pool.tile(
                            [128, MT8, CW], f8, name="expS8", tag="expS8"
                        )

                    def e_tile(mt):
                        if mt < MCUT:
                            return expS[:, mt, :]
                        return expS8[:, mt - MCUT, :]

                    dacc = None
                    for mt in range(NT):
                        ps = ps_scores.tile([128, CW], f32, name="ps_s", tag="scores")
                        for k in range(KT):
                            nc.tensor.matmul(
                                ps[:, :],
                                lhsT=xT[:, k, mt * 128 : (mt + 1) * 128],
                                rhs=qTs[c][:, k, :],
                                start=(k == 0),
                                stop=(k == KT - 1),
                            )
                        nc.scalar.activation(
                            e_tile(mt), ps[:, :],
                            mybir.ActivationFunctionType.Exp, scale=INV_SQRT_D,
                        )
                        # DVE running sum of exp tiles (fp16 partials; the
                        # cross-partition sum happens in f32 PSUM below)
                        if mt == 1:
                            dacc = dacc_pool.tile([128, CW], f16, name="dacc", tag="dacc")
                            nc.vector.tensor_add(dacc[:, :], e_tile(0), e_tile(1))
                        elif mt >= 2:
                            prev = dacc
                            dacc = dacc_pool.tile([128, CW], f16, name="dacc", tag="dacc")
                            nc.vector.tensor_add(dacc[:, :], prev[:, :], e_tile(mt))

                    # ---- AV + denominator broadcast + normalize ----
                    # bcast matmul emitted after AV dt1 so the PE never waits
                    # on the DVE den chain; the dt0/dt1 normalizes follow rcp
                    rcpB = rcp_pool.tile([128, CW], f32, name="rcpB", tag="rcpB")
                    ps_list = [None] * KT

                    def av_group(dt):
                        ps = ps_av.tile([128, CW], f32, name="ps_av", tag="av")
                        for mt in range(MCUT):
                            nc.tensor.matmul(
                                ps[:, :],
                                lhsT=qn[:, mt, dt * 128 : (dt + 1) * 128],
                                rhs=expS[:, mt, :],
                                start=(mt == 0),
                                stop=(MT8 == 0 and mt == NT - 1),
                            )
                        for p in range(MT8 // 2):
                            nc.tensor.matmul(
                                ps[:, :],
                                lhsT=qn8[:, 2 * p : 2 * p + 2, dt * 128 : (dt + 1) * 128],
                                rhs=expS8[:, 2 * p : 2 * p + 2, :],
                                start=False,
                                stop=(p == MT8 // 2 - 1),
                                perf_mode=DR,
                            )
                        ps_list[dt] = ps

                    def av_mul(dt):
                        nc.vector.tensor_mul(
                            outT[:, dt, n0 : n0 + CW], ps_list[dt][:, :], rcpB[:, :]
                        )

                    av_group(0)
                    av_group(1)
                    # sum den partials over partitions + broadcast
                    psb = ps_bcast.tile([128, CW], f32, name="psb", tag="bcast")
                    nc.tensor.matmul(
                        psb[:, :], lhsT=ones128[:, :], rhs=dacc[:, :],
                        start=True, stop=True,
                    )
                    nc.vector.reciprocal_approx_fast(rcpB[:, :], psb[:, :])
                    av_mul(0)
                    av_group(2)
                    av_mul(1)
                    av_group(3)
                    av_mul(2)
                    av_mul(3)

                    # ---- final projection, interleaved into the last head ----
                    if h == HPG - 1:
                        for nt in range(c * (CW // 128), (c + 1) * (CW // 128)):
                            ps = ps_stage.tile([128, D], f32, name="ps_y", tag="stage")
                            for hh in range(HPG):
                                for dtt in range(KT):
                                    jt = hh * KT + dtt
                                    nc.tensor.matmul(
                                        ps[:, :],
                                        lhsT=outTs[hh][:, dtt, nt * 128 : (nt + 1) * 128],
                                        rhs=wp[:, jt, :],
                                        start=(jt == 0),
                                        stop=(jt == HPG * KT - 1),
                                    )
                            ysb = y_pool.tile([128, D], f16, name="ysb", tag="y")
                            if nt % 2 == 0:
                                nc.scalar.copy(ysb[:, :], ps[:, :])
                            else:
                                nc.vector.tensor_copy(ysb[:, :], ps[:, :])
                            for u in range(2):
                                nc.sync.dma_start(
                                    y_d[nt * 128 : (nt + 1) * 128, u * 256 : (u + 1) * 256],
                                    ysb[:, u * 256 : (u + 1) * 256],
                                )

    nc.compile()
    return nc


def _ensure_nc():
    if "nc" not in _state:
        _state["nc"] = _build()
    return _state["nc"]


def _make_in_maps(x, Wq, Wp):
    f16 = np.float16
    in_maps = []
    for c in range(NCORES):
        b, hg = c // HG, c % HG
        xt16 = np.ascontiguousarray(x[b].T).astype(f16)
        m = {
            "xt": xt16,
            "wq": np.ascontiguousarray(Wq[:, hg * JW : (hg + 1) * JW]).astype(f16),
            "wp": np.ascontiguousarray(Wp[hg * JW : (hg + 1) * JW, :]).astype(f16),
        }
        in_maps.append(m)
    return in_maps


def _get_runner():
    """Build once and cache a jitted 8-core runner (avoids re-jit per call)."""
    if "run" in _state:
        return _state["run"]

    import jax
    import concourse.mybir as mybir
    from jax.sharding import Mesh, PartitionSpec
    from jax.experimental.shard_map import shard_map
    from concourse import bass2jax

    nc = _ensure_nc()
    bass2jax.install_neuronx_cc_hook()

    partition_name = nc.partition_id_tensor.name if nc.partition_id_tensor else None
    in_names, out_names, out_avals, zero_outs = [], [], [], []
    for alloc in nc.m.functions[0].allocations:
        if not isinstance(alloc, mybir.MemoryLocationSet):
            continue
        name = alloc.memorylocations[0].name
        if alloc.kind == "ExternalInput":
            if name != partition_name:
                in_names.append(name)
        elif alloc.kind == "ExternalOutput":
            shape = tuple(alloc.tensor_shape)
            dtype = mybir.dt.np(alloc.dtype)
            out_avals.append(jax.core.ShapedArray(shape, dtype))
            out_names.append(name)
            zero_outs.append(np.zeros(shape, dtype))
    n_params = len(in_names)
    n_outs = len(out_names)
    all_in_names = list(in_names) + list(out_names)
    if partition_name is not None:
        all_in_names.append(partition_name)

    def _body(*args):
        operands = list(args)
        if partition_name is not None:
            operands.append(bass2jax.partition_id_tensor())
        outs = bass2jax._bass_exec_p.bind(
            *operands,
            out_avals=tuple(out_avals),
            in_names=tuple(all_in_names),
            out_names=tuple(out_names),
            lowering_input_output_aliases=(),
            sim_require_finite=True,
            sim_require_nnan=True,
            nc=nc,
        )
        return tuple(outs)

    devices = jax.devices()[:NCORES]
    mesh = Mesh(np.asarray(devices), ("core",))
    in_specs = (PartitionSpec("core"),) * (n_params + n_outs)
    out_specs = (PartitionSpec("core"),) * n_outs
    sharded = jax.jit(
        shard_map(_body, mesh=mesh, in_specs=in_specs, out_specs=out_specs,
                  check_rep=False),
        donate_argnums=tuple(range(n_params, n_params + n_outs)),
        keep_unused=True,
    )

    def run(in_maps):
        concat_in = [
            np.concatenate([np.asarray(m[name]) for m in in_maps], axis=0)
            for name in in_names
        ]
        concat_zeros = [
            np.zeros((NCORES * z.shape[0], *z.shape[1:]), z.dtype) for z in zero_outs
        ]
        out_arrs = sharded(*concat_in, *concat_zeros)
        return [
            {
                name: np.asarray(out_arrs[i]).reshape(NCORES, *out_avals[i].shape)[c]
                for i, name in enumerate(out_names)
            }
            for c in range(NCORES)
        ]

    _state["run"] = run
    return run


def kernel(x, Wq, Wv, Wp, bp):
    x = np.asarray(x, np.float32)
    Wq = np.asarray(Wq, np.float32)
    Wp = np.asarray(Wp, np.float32)
    bp = np.asarray(bp, np.float32)

    run = _get_runner()
    results = run(_make_in_maps(x, Wq, Wp))
    y = np.empty((B, N, D), np.float32)
    for b in range(B):
        y[b] = (results[b * HG]["y"].astype(np.float32)
                + results[b * HG + 1]["y"].astype(np.float32)
                + bp[None, :])
    return y
